# revision 70
# baseline (speedup 1.0000x reference)
"""Trainium2 Bass kernel for nn_CausalFieldAttention (v2).

Shapes (hardcoded): B=4, N=4096, D=1024, H=16, hd=64, G=512, sigma=3.

Reference computation (q-projection is computed but unused -> skipped):
    k  = x @ k_w.T + k_b                      (B,N,D) -> heads (B,H,N,hd)
    v  = x @ v_w.T + v_b
    wv = v * ||k||_head
    field = segment_sum(wv, field_idx, G)     scatter tokens -> G bins
    conv  = circular_conv(field, causal_ker)  (exact circulant)
    y  = conv[field_idx]                      gather bins -> tokens
    out = y @ out_w.T + out_b

Device strategy: 8 cores = 4 batches x 2 head-groups (512 channels each).
v2 changes vs v1 (206-244us baseline):
  - Projections/scatter/conv operands in bf16: enables the PE's automatic
    fast-weight-load (FWL, off for fp32 modes), halving the per-matmul
    LDWEIGHTS tax, and halves all input DMA traffic.
  - out = gather(conv @ ow) where A := conv @ ow is computed at bin
    granularity; the gather is a pure row-replication (8 tokens per bin,
    seven 9-runs, one 1-run) done with ~19 affine DMAs straight from
    A in SBUF to DRAM -- no gather matmuls, no output staging copies.
  - Fine-grained dependency schedule: field bins complete monotonically
    with token index; conv[g] only needs field[g-255 .. g-176] (kernel
    support > 1e-12).  conv+A are computed per 32-aligned g-range as soon
    as the last contributing 64-bin field half-tile lands, and each
    range's output tokens stream to DRAM immediately.  Only conv bins
    ~[96,256) structurally depend on the last tokens => ~5MB tail instead
    of v1's ~half-output tail.
  - conv accumulated per-range in PSUM (not SBUF read-modify-write).
"""

import os
import sys
from contextlib import ExitStack

import numpy as np

for _p in ("/opt/trn_rl_repo", "/root/.axon_site/_ro/trn_rl_repo"):
    if os.path.isdir(_p) and _p not in sys.path:
        sys.path.append(_p)

import concourse.bacc as bacc
import concourse.mybir as mybir
import concourse.tile as tile
from concourse.bass_utils import run_bass_kernel_spmd

B, N, D = 4, 4096, 1024
H, HD, G = 16, 64, 512
SIGMA = 3.0
P = 128
KT = D // P          # 8 contraction tiles over D
TT = N // P          # 32 token tiles
GT = G // P          # 4 bin tiles
HB = 64              # bins per half-tile
NHALF = G // HB      # 8 half-tiles
CLOC = 512           # channels per core (8 heads)
HLOC = CLOC // HD    # 8 heads per core
ECH = D // 512       # 2 chunks of out-channels for 512-wide psum
NCORES = 8

F32 = mybir.dt.float32
F32R = mybir.dt.float32r
BF16 = mybir.dt.bfloat16
FP8 = mybir.dt.float8e4
NP_BF16 = mybir.dt.np(BF16)
NP_FP8 = mybir.dt.np(FP8)
KSCALE = 32.0   # k-weights are scaled x32 into fp8's normal range; the
                # resulting 32x on ||k|| is compensated exactly (power of
                # two) by scaling the conv matrix by 1/32.

# set by test harness to capture a profile; kernel() stores results here
TRACE = False
LAST_RESULT = None


def _field_idx():
    # exactly mirrors the reference (fp32 div then mul, trunc, clip)
    pos = np.arange(N, dtype=np.float32) / np.float32(N - 1) * np.float32(G - 1)
    return np.clip(pos.astype(np.int32), 0, G - 1)


def _causal_kernel():
    i = np.arange(G)
    dist = np.abs(i - G // 2)
    ker = np.where(i >= G // 2, 0.0, np.exp(-dist / SIGMA)).astype(np.float32)
    ker = ker / (ker.sum() + 1e-8)
    return ker


def _plans():
    idx = _field_idx()
    ker = _causal_kernel()
    gg = (np.arange(G)[None, :] - np.arange(G)[:, None]) % G
    CTm = ker[gg].astype(np.float32)      # CTm[f, g] = ker[(g-f)%G]

    Smat = np.zeros((N, G), np.float32)
    Smat[np.arange(N), idx] = 1.0

    # kernel support: ker[m] > 1e-12 for m in [mlo, 255]
    nz = np.where(ker > 1e-12)[0]
    mlo, mhi = int(nz.min()), int(nz.max())          # 176, 255

    counts = np.bincount(idx, minlength=G)           # tokens per bin
    tok_start = np.concatenate([[0], np.cumsum(counts)])

    # scatter jobs per token tile: (gt, half, hsl_lo, first, last) where
    # first/last flag whether this tile is the first/last contributor to
    # that 64-bin half (per-half PSUM accumulation groups).
    tile_halves = []
    for t in range(TT):
        bt = idx[t * P:(t + 1) * P]
        tile_halves.append(sorted(set((bt // HB).tolist())))
    half_tts = {h: [t for t in range(TT) if h in tile_halves[t]]
                for h in range(NHALF)}
    half_last = {h: max(half_tts[h]) for h in range(NHALF)}
    tile_gts = [sorted(set(h // 2 for h in hs)) for hs in tile_halves]

    # conv/A ranges (32-aligned, within one gt).  conv[g] needs field bins
    # [g-mhi, g-mlo] mod G.  Ready-half = the half-tile that completes last
    # among contributors (field completes in bin order).
    def range_halves(glo, ghi):
        hs = set()
        for h in range(NHALF):
            # contribution window of half h: [64h+mlo, 64h+63+mhi] mod G
            w0, w1 = h * HB + mlo, h * HB + HB - 1 + mhi
            for g in range(glo, ghi):
                gg_ = g if g >= w0 % G or True else g
                # membership test in the mod-G interval [w0, w1]
                if (g - w0) % G <= (w1 - w0):
                    hs.add(h)
                    break
        return sorted(hs)

    ranges = []
    # all matmul outputs are kept at partition base 0 (ISA rejects nonzero
    # dst partition offsets): A lives in a per-range layout.
    # conv[g] touches the final field half (bins 448+) only for g >= 112,
    # so [0,112) is ready at half-6 (token tile 28) and only [112,256)
    # is tail-blocked.
    for ri, (glo, ghi) in enumerate(
            ((0, 112), (112, 128), (128, 256), (256, 384), (384, 512))):
        hs = range_halves(glo, ghi)
        # trigger = the half among hs that completes last in token order.
        # field half h completes at token tile half_last[h]; completion
        # order of halves is simply 0,1,2,...,7.
        trig = max(hs, key=lambda h: half_last[h])
        # out-DMA chunks: (tok0, bin0, nbins, rep) with uniform rep
        chunks = []
        b = glo
        while b < ghi:
            c = int(counts[b])
            nb = 1
            while b + nb < ghi and int(counts[b + nb]) == c:
                nb += 1
            chunks.append((int(tok_start[b]), b, nb, c))
            b += nb
        ranges.append({
            "ri": ri, "glo": glo, "ghi": ghi, "halves": hs,
            "trigger_tile": half_last[trig], "chunks": chunks,
        })
    return {
        "idx": idx, "CTm": CTm, "Smat": Smat, "mlo": mlo, "mhi": mhi,
        "tile_halves": tile_halves, "tile_gts": tile_gts,
        "half_last": half_last, "ranges": ranges,
    }


def _build_program(with_kb, with_vb, pl):
    tile_halves = pl["tile_halves"]
    tile_gts = pl["tile_gts"]
    half_last = pl["half_last"]
    ranges = pl["ranges"]

    nc = bacc.Bacc("TRN2", target_bir_lowering=False, debug=False,
                   num_devices=NCORES)
    # host-permuted layouts: per-partition-contiguous so every DMA moves
    # >=2KB per descriptor row.
    xTt = nc.dram_tensor("xTt", [TT * P, KT * P], BF16, kind="ExternalInput").ap()
    x8t = nc.dram_tensor("x8t", [TT * P, KT * P], FP8, kind="ExternalInput").ap()
    kwt = nc.dram_tensor("kwt", [P, KT * CLOC], FP8, kind="ExternalInput").ap()
    vwt = nc.dram_tensor("vwt", [P, KT * CLOC], BF16, kind="ExternalInput").ap()
    owt = nc.dram_tensor("owt", [P, GT * D], BF16, kind="ExternalInput").ap()
    ctt = nc.dram_tensor("ctt", [HB, NHALF * G], BF16, kind="ExternalInput").ap()
    Sm = nc.dram_tensor("Smat", [N, G], BF16, kind="ExternalInput").ap()
    kb = nc.dram_tensor("kb", [1, CLOC], BF16, kind="ExternalInput").ap() if with_kb else None
    vb = nc.dram_tensor("vb", [1, CLOC], BF16, kind="ExternalInput").ap() if with_vb else None
    ones_d = (nc.dram_tensor("ones", [1, P], BF16, kind="ExternalInput").ap()
              if (with_kb or with_vb) else None)
    # device output: A = conv @ ow at bin granularity, one 128-row slab per
    # range.  The token gather out[t] = A[idx[t]] is pure row replication and
    # is done on the host during unshard (together with the partial sum).
    aout = nc.dram_tensor("aout", [len(ranges) * P, D], F32,
                          kind="ExternalOutput").ap()

    with tile.TileContext(nc) as tc, ExitStack() as es:
        cpool = es.enter_context(tc.tile_pool(name="const", bufs=1))

        NR = len(ranges)
        kw_sb = cpool.tile([P, KT, CLOC], FP8)
        vw_sb = cpool.tile([P, KT, CLOC], BF16)
        ow_sb = cpool.tile([P, GT, D], BF16)
        # per-half layouts on partitions 0..63 (keeps all matmul operand and
        # output base partitions at 0)
        ct_sb = cpool.tile([HB, NHALF, G], BF16)    # [f%64, f//64, g]
        field_sb = cpool.tile([HB, NHALF, CLOC], BF16)
        convT_sb = cpool.tile([P, GT, G], BF16)     # [ch%128, ch//128, g]
        convP_sb = cpool.tile([P, 2, GT * P], F32)  # tail-range conv partials
        A_sb = cpool.tile([P, NR, D], F32)          # [bin-glo(r), r, e]
        if with_kb or with_vb:
            ones_sb = cpool.tile([1, P], BF16)
            nc.sync.dma_start(ones_sb[:], ones_d[:])
        if with_kb:
            kb_sb = cpool.tile([1, CLOC], BF16)
            nc.sync.dma_start(kb_sb[:], kb[:])
        if with_vb:
            vb_sb = cpool.tile([1, CLOC], BF16)
            nc.sync.dma_start(vb_sb[:], vb[:])

        xpool = es.enter_context(tc.tile_pool(name="xin", bufs=4))
        x8pool = es.enter_context(tc.tile_pool(name="x8in", bufs=4))
        spool = es.enter_context(tc.tile_pool(name="sblk", bufs=6))
        smpool = es.enter_context(tc.tile_pool(name="small", bufs=3))
        wvpool = es.enter_context(tc.tile_pool(name="wv", bufs=3))
        # 4-deep k/v ring: v(t+1) reuses the slot freed by square(k(t+3)) on
        # ACT; at depth 3 a momentarily busy ACT stalls the PE v-stream.
        # The mid pool tolerates depth 2 (its copies drain within ~0.5us).
        ps_kv = es.enter_context(tc.tile_pool(name="ps_kv", bufs=4, space="PSUM"))
        ps_f = es.enter_context(tc.tile_pool(name="ps_f", bufs=2, space="PSUM"))
        ps_m = es.enter_context(tc.tile_pool(name="ps_m", bufs=2, space="PSUM"))

        kwt_r = kwt.rearrange("p (kt c) -> p kt c", kt=KT)
        vwt_r = vwt.rearrange("p (kt c) -> p kt c", kt=KT)

        field_ps = {}
        s_tiles = {}
        eng_flip = [0]

        def flip_copy(dst, src):
            # alternate DVE/ACT for PSUM->SBUF traffic
            if eng_flip[0] % 2 == 0:
                nc.vector.tensor_copy(dst, src)
            else:
                nc.scalar.copy(dst, src)
            eng_flip[0] += 1

        def emit_scatter(t, wv):
            tsl = slice(t * P, (t + 1) * P)
            for h in tile_halves[t]:
                gt = h // 2
                hsl = slice((h % 2) * HB, (h % 2) * HB + HB)
                first = (t == min(tt for tt in range(TT) if h in tile_halves[tt]))
                last = (t == half_last[h])
                if (t, gt) not in s_tiles:
                    st = spool.tile([P, P], BF16, tag="sblk")
                    nc.gpsimd.dma_start(st[:], Sm[tsl, gt * P:(gt + 1) * P])
                    s_tiles[(t, gt)] = st
                if h not in field_ps:
                    field_ps[h] = ps_f.tile([HB, CLOC], F32, tag="fld",
                                            name=f"fld{h}")
                nc.tensor.matmul(field_ps[h][:],
                                 s_tiles[(t, gt)][:, hsl],
                                 wv[:], start=first, stop=last)
                if last:
                    if h == NHALF - 1:
                        # tail-critical copy: split across both engines so
                        # the fin conv matmuls (ct 0-1 first) start sooner
                        nc.vector.tensor_copy(field_sb[:, h, 0:CLOC // 2],
                                              field_ps[h][:, 0:CLOC // 2])
                        nc.scalar.copy(field_sb[:, h, CLOC // 2:CLOC],
                                       field_ps[h][:, CLOC // 2:CLOC])
                    else:
                        flip_copy(field_sb[:, h, :], field_ps[h][:])
                    del field_ps[h]

        def conv_mms(r, halves):
            glo, ghi = r["glo"], r["ghi"]
            W = ghi - glo
            cv = ps_m.tile([P, 512], F32, tag="mid")
            for ct in range(GT):
                for j, h in enumerate(halves):
                    nc.tensor.matmul(
                        cv[:, ct * W:(ct + 1) * W],
                        field_sb[:, h, ct * P:(ct + 1) * P],
                        ct_sb[:, h, glo:ghi],
                        start=(j == 0), stop=(j == len(halves) - 1))
            return cv

        def job_A(r):
            ri, glo, ghi = r["ri"], r["glo"], r["ghi"]
            W = ghi - glo
            for ec in range(ECH):
                esl = slice(ec * 512, (ec + 1) * 512)
                pa = ps_m.tile([P, 512], F32, tag="mid")
                for ct in range(GT):
                    nc.tensor.matmul(pa[0:W, :],
                                     convT_sb[:, ct, glo:ghi],
                                     ow_sb[:, ct, esl],
                                     start=(ct == 0), stop=(ct == GT - 1))
                flip_copy(A_sb[0:W, ri, esl], pa[0:W, :])
                eng = nc.sync if ec == 0 else nc.scalar
                eng.dma_start(aout[ri * P:ri * P + W, esl], A_sb[0:W, ri, esl])

        def job_range(r):
            # conv (all halves) -> convT bf16, then A + out
            cv = conv_mms(r, r["halves"])
            glo, ghi = r["glo"], r["ghi"]
            W = ghi - glo
            for lo, hi in ((0, 2), (2, 4)):
                flip_copy(convT_sb[:, lo:hi, glo:ghi],
                          cv[:, lo * W:hi * W].rearrange("p (ct w) -> p ct w", w=W))
            job_A(r)

        def job_range_pre(r, slot):
            # tail ranges: pre-accumulate every half but the last into an
            # f32 scratch so the critical tail only runs the h7 matmuls
            cv = conv_mms(r, [h for h in r["halves"] if h != NHALF - 1])
            W = r["ghi"] - r["glo"]
            flip_copy(convP_sb[:, slot, 0:GT * W],
                      cv[:, 0:GT * W])

        def fin_add(r, slot, cv):
            glo, ghi = r["glo"], r["ghi"]
            W = ghi - glo
            for lo, hi in ((0, 2), (2, 4)):
                nc.vector.tensor_tensor(
                    convT_sb[:, lo:hi, glo:ghi],
                    cv[:, lo * W:hi * W].rearrange("p (ct w) -> p ct w", w=W),
                    convP_sb[:, slot, lo * W:hi * W]
                    .rearrange("p (ct w) -> p ct w", w=W),
                    mybir.AluOpType.add)

        # non-tail ranges run fully at their trigger tile; tail ranges
        # (trigger == last tile) pre-accumulate early halves at tile TT-3
        # and only run the last half + A after the final scatter.
        jobs_at = {}
        tail_jobs = []
        for r in ranges:
            if r["trigger_tile"] == TT - 1:
                slot = len(tail_jobs)
                tail_jobs.append((r, slot))
                jobs_at.setdefault(TT - 3, []).append(
                    ("pre", r, slot))
            else:
                jobs_at.setdefault(r["trigger_tile"], []).append(
                    ("full", r, None))

        # ---- startup DMA plan: three queues, deadline-ordered ----
        # The PE runs the (cheap-operand) fp8 k-projection LEAD tiles ahead
        # of the v-projection, so only x8/kw8 (~0.7MB) gates the PE start
        # while the 1MB vw streams in behind it.
        LEAD = 4
        xb_pre = {t: xpool.tile([P, KT, P], BF16, tag="xblk", bufs=4,
                                name=f"xb{t}") for t in range(2)}
        x8_pre = {t: x8pool.tile([P, KT, P], FP8, tag="x8blk", bufs=LEAD + 3,
                                 name=f"x8_{t}") for t in range(LEAD + 2)}

        def kwp(j):
            return (kw_sb[:, 2 * j:2 * j + 2, :], kwt_r[:, 2 * j:2 * j + 2, :])
        def vwp(j):
            return (vw_sb[:, 2 * j:2 * j + 2, :], vwt_r[:, 2 * j:2 * j + 2, :])
        def xbf(t):
            return (xb_pre[t][:], xTt[t * P:(t + 1) * P, :]
                    .rearrange("p (kt c) -> p kt c", kt=KT))
        def x8f(t):
            return (x8_pre[t][:], x8t[t * P:(t + 1) * P, :]
                    .rearrange("p (kt c) -> p kt c", kt=KT))
        def sblk(t0, gt):
            st = spool.tile([P, P], BF16, tag="sblk")
            s_tiles[(t0, gt)] = st
            return (st[:], Sm[t0 * P:(t0 + 1) * P, gt * P:(gt + 1) * P])
        plan = {
            nc.sync:   [x8f(0), kwp(2), x8f(2), vwp(1), x8f(4), xbf(1)],
            nc.scalar: [kwp(0), kwp(3), x8f(3), vwp(2), vwp(3), x8f(5)],
            nc.gpsimd: [kwp(1), x8f(1), xbf(0), vwp(0), sblk(0, 0), sblk(1, 0)],
        }
        for eng, items in plan.items():
            for dst, srcap in items:
                eng.dma_start(dst, srcap)

        xb_tiles = dict(xb_pre)
        x8_tiles = dict(x8_pre)
        ksq_tiles = {}
        pending = None

        def emit_k(tk, x8):
            kps = ps_kv.tile([P, CLOC], F32, tag="kv", name="kps")
            for j in range(KT // 2):
                nc.tensor.matmul(kps[:], x8[:, 2 * j:2 * j + 2, :],
                                 kw_sb[:, 2 * j:2 * j + 2, :],
                                 perf_mode=mybir.MatmulPerfMode.DoubleRow,
                                 start=(j == 0),
                                 stop=(j == KT // 2 - 1 and not with_kb))
            if with_kb:
                nc.tensor.matmul(kps[:], ones_sb[:], kb_sb[:], start=False, stop=True)
            # square immediately: frees the PSUM slot early and decouples
            # the ||k|| chain from the k/v PE cadence
            ksq = smpool.tile([P, CLOC], F32, tag="ksq", bufs=LEAD + 2)
            nc.scalar.activation(ksq[:], kps[:], mybir.ActivationFunctionType.Square)
            ksq_tiles[tk] = ksq

        km_tiles = {}

        def emit_km(t):
            # ||k|| for tile t, one iteration ahead of its use so wv(t)
            # fires the moment vps(t) stops (matters at the tail entry)
            ksq = ksq_tiles.pop(t)
            km2 = smpool.tile([P, HLOC], F32, tag="km2")
            nc.vector.reduce_sum(km2[:], ksq[:].rearrange("p (h d) -> p h d", d=HD),
                                 axis=mybir.AxisListType.X)
            km = smpool.tile([P, HLOC], F32, tag="km")
            nc.scalar.sqrt(km[:], km2[:])
            km_tiles[t] = km

        # k prologue: tiles 0..LEAD-1, plus tile 0's ||k||
        for tk in range(LEAD):
            emit_k(tk, x8_tiles.pop(tk))
        emit_km(0)

        for t in range(TT):
            tk = t + LEAD
            # prefetches: x8 for tile t+LEAD+2, xb for tile t+2
            tn8 = t + LEAD + 2
            if tn8 < TT and tn8 not in x8_tiles:
                x8n = x8pool.tile([P, KT, P], FP8, tag="x8blk", bufs=LEAD + 3,
                                  name="x8")
                nc.sync.dma_start(x8n[:], x8t[tn8 * P:(tn8 + 1) * P, :]
                                  .rearrange("p (kt c) -> p kt c", kt=KT))
                x8_tiles[tn8] = x8n
            tn = t + 2
            if tn < TT and tn not in xb_tiles:
                xbn = xpool.tile([P, KT, P], BF16, tag="xblk", bufs=4, name="xb")
                nc.sync.dma_start(xbn[:], xTt[tn * P:(tn + 1) * P, :]
                                  .rearrange("p (kt c) -> p kt c", kt=KT))
                xb_tiles[tn] = xbn
            if t == 8:
                nc.scalar.dma_start(ct_sb[:], ctt.rearrange(
                    "p (h g) -> p h g", h=NHALF))
            if t == 12:
                nc.scalar.dma_start(ow_sb[:], owt.rearrange(
                    "p (gt e) -> p gt e", gt=GT))
            # prefetch S blocks two tiles ahead
            tp = t + 2
            if tp < TT:
                tsl = slice(tp * P, (tp + 1) * P)
                for gt in tile_gts[tp]:
                    if (tp, gt) not in s_tiles:
                        st = spool.tile([P, P], BF16, tag="sblk")
                        nc.gpsimd.dma_start(st[:], Sm[tsl, gt * P:(gt + 1) * P])
                        s_tiles[(tp, gt)] = st

            if tk < TT:
                emit_k(tk, x8_tiles.pop(tk))

            xb = xb_tiles.pop(t)
            vps = ps_kv.tile([P, CLOC], F32, tag="kv", name="vps")
            for kt in range(KT):
                nc.tensor.matmul(vps[:], xb[:, kt, :], vw_sb[:, kt, :],
                                 start=(kt == 0), stop=(kt == KT - 1 and not with_vb))
            if with_vb:
                nc.tensor.matmul(vps[:], ones_sb[:], vb_sb[:], start=False, stop=True)

            # scatter of the previous tile (its wv is ready by now)
            if pending is not None:
                emit_scatter(*pending)
                for kind, r, slot in jobs_at.get(pending[0], []):
                    if kind == "pre":
                        job_range_pre(r, slot)
                    else:
                        job_range(r)

            if t + 1 < TT:
                emit_km(t + 1)

            # wv = v * ||k|| -> bf16  (km precomputed last iteration)
            km = km_tiles.pop(t)
            wv = wvpool.tile([P, CLOC], BF16, tag="wv")
            nc.vector.tensor_tensor(
                wv[:].rearrange("p (h d) -> p h d", d=HD),
                vps[:].rearrange("p (h d) -> p h d", d=HD),
                km[:].unsqueeze(2).broadcast_to((P, HLOC, HD)),
                mybir.AluOpType.mult)
            pending = (t, wv)

        emit_scatter(*pending)
        # interleave the two tail ranges: both ranges' conv matmuls issue
        # back-to-back so each range's DVE accumulate hides under the other
        # range's PE work; big range first so the drain-gating final DMA is
        # the small one.
        fins = sorted(tail_jobs, key=lambda rs: -(rs[0]["ghi"] - rs[0]["glo"]))
        cvs = [(r, slot, conv_mms(r, [NHALF - 1])) for r, slot in fins]
        for r, slot, cv in cvs:
            fin_add(r, slot, cv)
        for r, slot, cv in cvs:
            job_A(r)

    nc.compile()
    return nc


_PROGRAM_CACHE = {}
_PLANS_CACHE = {}


def _get_plans():
    if "p" not in _PLANS_CACHE:
        _PLANS_CACHE["p"] = _plans()
    return _PLANS_CACHE["p"]


def _get_program(with_kb, with_vb):
    key = (with_kb, with_vb)
    if key not in _PROGRAM_CACHE:
        _PROGRAM_CACHE[key] = _build_program(with_kb, with_vb, _get_plans())
    return _PROGRAM_CACHE[key]


def kernel(x, q_w, q_b, k_w, k_b, v_w, v_b, out_w, out_b):
    global LAST_RESULT
    x = np.asarray(x, dtype=np.float32)
    k_w = np.asarray(k_w, dtype=np.float32)
    k_b = np.asarray(k_b, dtype=np.float32)
    v_w = np.asarray(v_w, dtype=np.float32)
    v_b = np.asarray(v_b, dtype=np.float32)
    out_w = np.asarray(out_w, dtype=np.float32)
    out_b = np.asarray(out_b, dtype=np.float32)

    with_kb = bool(np.any(k_b))
    with_vb = bool(np.any(v_b))
    nc = _get_program(with_kb, with_vb)
    pl = _get_plans()
    Smat = pl["Smat"].astype(NP_BF16)
    # 1/KSCALE compensates the x KSCALE on the fp8 k-weights (exact: the
    # bf16 CT values just shift exponent by 5)
    CTm = pl["CTm"] * np.float32(1.0 / KSCALE)
    # ctt[p, h*G+g] = CTm[h*64+p, g]
    ctt = np.ascontiguousarray(
        CTm.reshape(NHALF, HB, G).transpose(1, 0, 2).reshape(HB, NHALF * G)
    ).astype(NP_BF16)

    in_maps = []
    for c in range(NCORES):
        b, hg = c // 2, c % 2
        chs = slice(hg * CLOC, (hg + 1) * CLOC)
        # xTt[t*128+p, kt*128+c] = x[b][t*128+c, kt*128+p]
        xb = x[b].reshape(TT, P, KT, P).transpose(0, 3, 2, 1) \
            .reshape(TT * P, KT * P)
        # kwt[p, kt*CLOC+ch] = KSCALE * k_w[chs][ch, kt*128+p]  (fp8)
        kwl = (k_w[chs, :].T * np.float32(KSCALE)) \
            .reshape(KT, P, CLOC).transpose(1, 0, 2).reshape(P, KT * CLOC)
        vwl = v_w[chs, :].T.reshape(KT, P, CLOC).transpose(1, 0, 2) \
            .reshape(P, KT * CLOC)
        # owt[p, ct*D+e] = out_w[e, ct*128+p(within chs)]
        owl = out_w[:, chs].T.reshape(GT, P, D).transpose(1, 0, 2) \
            .reshape(P, GT * D)
        m = {
            "xTt": np.ascontiguousarray(xb).astype(NP_BF16),
            "x8t": np.ascontiguousarray(xb).astype(NP_FP8),
            "kwt": np.ascontiguousarray(kwl).astype(NP_FP8),
            "vwt": np.ascontiguousarray(vwl).astype(NP_BF16),
            "owt": np.ascontiguousarray(owl).astype(NP_BF16),
            "ctt": ctt,
            "Smat": Smat,
        }
        if with_kb:
            # k is computed scaled by KSCALE on device; scale the bias too
            m["kb"] = np.ascontiguousarray(
                k_b[chs][None, :] * np.float32(KSCALE)).astype(NP_BF16)
        if with_vb:
            m["vb"] = np.ascontiguousarray(v_b[chs][None, :]).astype(NP_BF16)
        if with_kb or with_vb:
            m["ones"] = np.ones((1, P), dtype=NP_BF16)
        in_maps.append(m)

    res = run_bass_kernel_spmd(nc, in_maps, core_ids=list(range(NCORES)),
                               trace=TRACE)
    LAST_RESULT = res

    idx = pl["idx"]
    out = np.empty((B, N, D), dtype=np.float32)
    for b in range(B):
        # unshard: sum the two head-group partials of A, then replicate
        # bin rows out to tokens (pure gather) and add the output bias.
        A = np.zeros((G, D), dtype=np.float32)
        for part in (res.results[2 * b]["aout"], res.results[2 * b + 1]["aout"]):
            for r in pl["ranges"]:
                ri, glo, ghi = r["ri"], r["glo"], r["ghi"]
                A[glo:ghi] += part[ri * P:ri * P + (ghi - glo)]
        out[b] = A[idx]
        out[b] += out_b[None, :]
    return out


# revision 73
# speedup vs baseline: 1.0158x; 1.0158x over previous
"""Trainium2 Bass kernel for nn_CausalFieldAttention (v2).

Shapes (hardcoded): B=4, N=4096, D=1024, H=16, hd=64, G=512, sigma=3.

Reference computation (q-projection is computed but unused -> skipped):
    k  = x @ k_w.T + k_b                      (B,N,D) -> heads (B,H,N,hd)
    v  = x @ v_w.T + v_b
    wv = v * ||k||_head
    field = segment_sum(wv, field_idx, G)     scatter tokens -> G bins
    conv  = circular_conv(field, causal_ker)  (exact circulant)
    y  = conv[field_idx]                      gather bins -> tokens
    out = y @ out_w.T + out_b

Device strategy: 8 cores = 4 batches x 2 head-groups (512 channels each).
v2 changes vs v1 (206-244us baseline):
  - Projections/scatter/conv operands in bf16: enables the PE's automatic
    fast-weight-load (FWL, off for fp32 modes), halving the per-matmul
    LDWEIGHTS tax, and halves all input DMA traffic.
  - out = gather(conv @ ow) where A := conv @ ow is computed at bin
    granularity; the gather is a pure row-replication (8 tokens per bin,
    seven 9-runs, one 1-run) done with ~19 affine DMAs straight from
    A in SBUF to DRAM -- no gather matmuls, no output staging copies.
  - Fine-grained dependency schedule: field bins complete monotonically
    with token index; conv[g] only needs field[g-255 .. g-176] (kernel
    support > 1e-12).  conv+A are computed per 32-aligned g-range as soon
    as the last contributing 64-bin field half-tile lands, and each
    range's output tokens stream to DRAM immediately.  Only conv bins
    ~[96,256) structurally depend on the last tokens => ~5MB tail instead
    of v1's ~half-output tail.
  - conv accumulated per-range in PSUM (not SBUF read-modify-write).
"""

import os
import sys
from contextlib import ExitStack

import numpy as np

for _p in ("/opt/trn_rl_repo", "/root/.axon_site/_ro/trn_rl_repo"):
    if os.path.isdir(_p) and _p not in sys.path:
        sys.path.append(_p)

import concourse.bacc as bacc
import concourse.mybir as mybir
import concourse.tile as tile
from concourse.bass_utils import run_bass_kernel_spmd

B, N, D = 4, 4096, 1024
H, HD, G = 16, 64, 512
SIGMA = 3.0
P = 128
KT = D // P          # 8 contraction tiles over D
TT = N // P          # 32 token tiles
GT = G // P          # 4 bin tiles
HB = 64              # bins per half-tile
NHALF = G // HB      # 8 half-tiles
CLOC = 512           # channels per core (8 heads)
HLOC = CLOC // HD    # 8 heads per core
ECH = D // 512       # 2 chunks of out-channels for 512-wide psum
NCORES = 8

F32 = mybir.dt.float32
F32R = mybir.dt.float32r
BF16 = mybir.dt.bfloat16
FP8 = mybir.dt.float8e4
NP_BF16 = mybir.dt.np(BF16)
NP_FP8 = mybir.dt.np(FP8)
KSCALE = 32.0   # k-weights are scaled x32 into fp8's normal range; the
                # resulting 32x on ||k|| is compensated exactly (power of
                # two) by scaling the conv matrix by 1/32.

# set by test harness to capture a profile; kernel() stores results here
TRACE = False
LAST_RESULT = None


def _field_idx():
    # exactly mirrors the reference (fp32 div then mul, trunc, clip)
    pos = np.arange(N, dtype=np.float32) / np.float32(N - 1) * np.float32(G - 1)
    return np.clip(pos.astype(np.int32), 0, G - 1)


def _causal_kernel():
    i = np.arange(G)
    dist = np.abs(i - G // 2)
    ker = np.where(i >= G // 2, 0.0, np.exp(-dist / SIGMA)).astype(np.float32)
    ker = ker / (ker.sum() + 1e-8)
    return ker


def _plans():
    idx = _field_idx()
    ker = _causal_kernel()
    gg = (np.arange(G)[None, :] - np.arange(G)[:, None]) % G
    CTm = ker[gg].astype(np.float32)      # CTm[f, g] = ker[(g-f)%G]

    Smat = np.zeros((N, G), np.float32)
    Smat[np.arange(N), idx] = 1.0

    # kernel support: ker[m] > 1e-12 for m in [mlo, 255]
    nz = np.where(ker > 1e-12)[0]
    mlo, mhi = int(nz.min()), int(nz.max())          # 176, 255

    counts = np.bincount(idx, minlength=G)           # tokens per bin
    tok_start = np.concatenate([[0], np.cumsum(counts)])

    # scatter jobs per token tile: (gt, half, hsl_lo, first, last) where
    # first/last flag whether this tile is the first/last contributor to
    # that 64-bin half (per-half PSUM accumulation groups).
    tile_halves = []
    for t in range(TT):
        bt = idx[t * P:(t + 1) * P]
        tile_halves.append(sorted(set((bt // HB).tolist())))
    half_tts = {h: [t for t in range(TT) if h in tile_halves[t]]
                for h in range(NHALF)}
    half_last = {h: max(half_tts[h]) for h in range(NHALF)}
    tile_gts = [sorted(set(h // 2 for h in hs)) for hs in tile_halves]

    # conv/A ranges (32-aligned, within one gt).  conv[g] needs field bins
    # [g-mhi, g-mlo] mod G.  Ready-half = the half-tile that completes last
    # among contributors (field completes in bin order).
    def range_halves(glo, ghi):
        hs = set()
        for h in range(NHALF):
            # contribution window of half h: [64h+mlo, 64h+63+mhi] mod G
            w0, w1 = h * HB + mlo, h * HB + HB - 1 + mhi
            for g in range(glo, ghi):
                gg_ = g if g >= w0 % G or True else g
                # membership test in the mod-G interval [w0, w1]
                if (g - w0) % G <= (w1 - w0):
                    hs.add(h)
                    break
        return sorted(hs)

    ranges = []
    # all matmul outputs are kept at partition base 0 (ISA rejects nonzero
    # dst partition offsets): A lives in a per-range layout.
    # conv[g] touches the final field half (bins 448+) only for g >= 112,
    # so [0,112) is ready at half-6 (token tile 28) and only [112,256)
    # is tail-blocked.
    for ri, (glo, ghi) in enumerate(
            ((0, 112), (112, 128), (128, 256), (256, 384), (384, 512))):
        hs = range_halves(glo, ghi)
        # trigger = the half among hs that completes last in token order.
        # field half h completes at token tile half_last[h]; completion
        # order of halves is simply 0,1,2,...,7.
        trig = max(hs, key=lambda h: half_last[h])
        # out-DMA chunks: (tok0, bin0, nbins, rep) with uniform rep
        chunks = []
        b = glo
        while b < ghi:
            c = int(counts[b])
            nb = 1
            while b + nb < ghi and int(counts[b + nb]) == c:
                nb += 1
            chunks.append((int(tok_start[b]), b, nb, c))
            b += nb
        ranges.append({
            "ri": ri, "glo": glo, "ghi": ghi, "halves": hs,
            "trigger_tile": half_last[trig], "chunks": chunks,
        })
    return {
        "idx": idx, "CTm": CTm, "Smat": Smat, "mlo": mlo, "mhi": mhi,
        "tile_halves": tile_halves, "tile_gts": tile_gts,
        "half_last": half_last, "ranges": ranges,
    }


def _build_program(with_kb, with_vb, pl):
    tile_halves = pl["tile_halves"]
    tile_gts = pl["tile_gts"]
    half_last = pl["half_last"]
    ranges = pl["ranges"]

    nc = bacc.Bacc("TRN2", target_bir_lowering=False, debug=False,
                   num_devices=NCORES)
    # host-permuted layouts: per-partition-contiguous so every DMA moves
    # >=2KB per descriptor row.
    xTt = nc.dram_tensor("xTt", [TT * P, KT * P], BF16, kind="ExternalInput").ap()
    x8t = nc.dram_tensor("x8t", [TT * P, KT * P], FP8, kind="ExternalInput").ap()
    kwt = nc.dram_tensor("kwt", [P, KT * CLOC], FP8, kind="ExternalInput").ap()
    vwt = nc.dram_tensor("vwt", [P, KT * CLOC], BF16, kind="ExternalInput").ap()
    owt = nc.dram_tensor("owt", [P, GT * D], BF16, kind="ExternalInput").ap()
    ctt = nc.dram_tensor("ctt", [HB, NHALF * G], BF16, kind="ExternalInput").ap()
    Sm = nc.dram_tensor("Smat", [N, G], BF16, kind="ExternalInput").ap()
    kb = nc.dram_tensor("kb", [1, CLOC], BF16, kind="ExternalInput").ap() if with_kb else None
    vb = nc.dram_tensor("vb", [1, CLOC], BF16, kind="ExternalInput").ap() if with_vb else None
    ones_d = (nc.dram_tensor("ones", [1, P], BF16, kind="ExternalInput").ap()
              if (with_kb or with_vb) else None)
    # device output: A = conv @ ow at bin granularity, one 128-row slab per
    # range.  The token gather out[t] = A[idx[t]] is pure row replication and
    # is done on the host during unshard (together with the partial sum).
    aout = nc.dram_tensor("aout", [len(ranges) * P, D], F32,
                          kind="ExternalOutput").ap()

    with tile.TileContext(nc) as tc, ExitStack() as es:
        cpool = es.enter_context(tc.tile_pool(name="const", bufs=1))

        NR = len(ranges)
        kw_sb = cpool.tile([P, KT, CLOC], FP8)
        vw_sb = cpool.tile([P, KT, CLOC], BF16)
        ow_sb = cpool.tile([P, GT, D], BF16)
        # per-half layouts on partitions 0..63 (keeps all matmul operand and
        # output base partitions at 0)
        ct_sb = cpool.tile([HB, NHALF, G], BF16)    # [f%64, f//64, g]
        field_sb = cpool.tile([HB, NHALF, CLOC], BF16)
        convT_sb = cpool.tile([P, GT, G], BF16)     # [ch%128, ch//128, g]
        convP_sb = cpool.tile([P, 2, GT * P], F32)  # tail-range conv partials
        A_sb = cpool.tile([P, NR, D], F32)          # [bin-glo(r), r, e]
        if with_kb or with_vb:
            ones_sb = cpool.tile([1, P], BF16)
            nc.sync.dma_start(ones_sb[:], ones_d[:])
        if with_kb:
            kb_sb = cpool.tile([1, CLOC], BF16)
            nc.sync.dma_start(kb_sb[:], kb[:])
        if with_vb:
            vb_sb = cpool.tile([1, CLOC], BF16)
            nc.sync.dma_start(vb_sb[:], vb[:])

        xpool = es.enter_context(tc.tile_pool(name="xin", bufs=4))
        x8pool = es.enter_context(tc.tile_pool(name="x8in", bufs=4))
        spool = es.enter_context(tc.tile_pool(name="sblk", bufs=6))
        smpool = es.enter_context(tc.tile_pool(name="small", bufs=3))
        wvpool = es.enter_context(tc.tile_pool(name="wv", bufs=3))
        # 4-deep k/v ring: v(t+1) reuses the slot freed by square(k(t+3)) on
        # ACT; at depth 3 a momentarily busy ACT stalls the PE v-stream.
        # The mid pool tolerates depth 2 (its copies drain within ~0.5us).
        ps_kv = es.enter_context(tc.tile_pool(name="ps_kv", bufs=4, space="PSUM"))
        ps_f = es.enter_context(tc.tile_pool(name="ps_f", bufs=2, space="PSUM"))
        ps_m = es.enter_context(tc.tile_pool(name="ps_m", bufs=2, space="PSUM"))

        kwt_r = kwt.rearrange("p (kt c) -> p kt c", kt=KT)
        vwt_r = vwt.rearrange("p (kt c) -> p kt c", kt=KT)

        field_ps = {}
        s_tiles = {}
        eng_flip = [0]

        def flip_copy(dst, src):
            # alternate DVE/ACT for PSUM->SBUF traffic
            if eng_flip[0] % 2 == 0:
                nc.vector.tensor_copy(dst, src)
            else:
                nc.scalar.copy(dst, src)
            eng_flip[0] += 1

        def emit_scatter(t, wv):
            tsl = slice(t * P, (t + 1) * P)
            for h in tile_halves[t]:
                gt = h // 2
                hsl = slice((h % 2) * HB, (h % 2) * HB + HB)
                first = (t == min(tt for tt in range(TT) if h in tile_halves[tt]))
                last = (t == half_last[h])
                if (t, gt) not in s_tiles:
                    st = spool.tile([P, P], BF16, tag="sblk")
                    nc.gpsimd.dma_start(st[:], Sm[tsl, gt * P:(gt + 1) * P])
                    s_tiles[(t, gt)] = st
                if h not in field_ps:
                    field_ps[h] = ps_f.tile([HB, CLOC], F32, tag="fld",
                                            name=f"fld{h}")
                nc.tensor.matmul(field_ps[h][:],
                                 s_tiles[(t, gt)][:, hsl],
                                 wv[:], start=first, stop=last)
                if last:
                    flip_copy(field_sb[:, h, :], field_ps[h][:])
                    del field_ps[h]

        def conv_mms(r, halves):
            glo, ghi = r["glo"], r["ghi"]
            W = ghi - glo
            cv = ps_m.tile([P, 512], F32, tag="mid")
            for ct in range(GT):
                for j, h in enumerate(halves):
                    nc.tensor.matmul(
                        cv[:, ct * W:(ct + 1) * W],
                        field_sb[:, h, ct * P:(ct + 1) * P],
                        ct_sb[:, h, glo:ghi],
                        start=(j == 0), stop=(j == len(halves) - 1))
            return cv

        def job_A(r):
            ri, glo, ghi = r["ri"], r["glo"], r["ghi"]
            W = ghi - glo
            for ec in range(ECH):
                esl = slice(ec * 512, (ec + 1) * 512)
                pa = ps_m.tile([P, 512], F32, tag="mid")
                for ct in range(GT):
                    nc.tensor.matmul(pa[0:W, :],
                                     convT_sb[:, ct, glo:ghi],
                                     ow_sb[:, ct, esl],
                                     start=(ct == 0), stop=(ct == GT - 1))
                flip_copy(A_sb[0:W, ri, esl], pa[0:W, :])
                eng = nc.sync if ec == 0 else nc.scalar
                eng.dma_start(aout[ri * P:ri * P + W, esl], A_sb[0:W, ri, esl])

        def job_range(r):
            # conv (all halves) -> convT bf16, then A + out
            cv = conv_mms(r, r["halves"])
            glo, ghi = r["glo"], r["ghi"]
            W = ghi - glo
            for lo, hi in ((0, 2), (2, 4)):
                flip_copy(convT_sb[:, lo:hi, glo:ghi],
                          cv[:, lo * W:hi * W].rearrange("p (ct w) -> p ct w", w=W))
            job_A(r)

        def job_range_pre(r, slot):
            # tail ranges: pre-accumulate every half but the last into an
            # f32 scratch so the critical tail only runs the h7 matmuls
            cv = conv_mms(r, [h for h in r["halves"] if h != NHALF - 1])
            W = r["ghi"] - r["glo"]
            flip_copy(convP_sb[:, slot, 0:GT * W],
                      cv[:, 0:GT * W])

        def fin_add(r, slot, cv):
            glo, ghi = r["glo"], r["ghi"]
            W = ghi - glo
            for lo, hi in ((0, 2), (2, 4)):
                nc.vector.tensor_tensor(
                    convT_sb[:, lo:hi, glo:ghi],
                    cv[:, lo * W:hi * W].rearrange("p (ct w) -> p ct w", w=W),
                    convP_sb[:, slot, lo * W:hi * W]
                    .rearrange("p (ct w) -> p ct w", w=W),
                    mybir.AluOpType.add)

        # non-tail ranges run fully at their trigger tile; tail ranges
        # (trigger == last tile) pre-accumulate early halves at tile TT-3
        # and only run the last half + A after the final scatter.
        jobs_at = {}
        tail_jobs = []
        for r in ranges:
            if r["trigger_tile"] == TT - 1:
                slot = len(tail_jobs)
                tail_jobs.append((r, slot))
                jobs_at.setdefault(TT - 3, []).append(
                    ("pre", r, slot))
            else:
                jobs_at.setdefault(r["trigger_tile"], []).append(
                    ("full", r, None))

        # ---- startup DMA plan: three queues, deadline-ordered ----
        # The PE runs the (cheap-operand) fp8 k-projection LEAD tiles ahead
        # of the v-projection, so only x8/kw8 (~0.7MB) gates the PE start
        # while the 1MB vw streams in behind it.
        LEAD = 4
        xb_pre = {t: xpool.tile([P, KT, P], BF16, tag="xblk", bufs=4,
                                name=f"xb{t}") for t in range(2)}
        x8_pre = {t: x8pool.tile([P, KT, P], FP8, tag="x8blk", bufs=LEAD + 3,
                                 name=f"x8_{t}") for t in range(LEAD + 2)}

        def kwp(j):
            return (kw_sb[:, 2 * j:2 * j + 2, :], kwt_r[:, 2 * j:2 * j + 2, :])
        def vwp(j):
            return (vw_sb[:, 2 * j:2 * j + 2, :], vwt_r[:, 2 * j:2 * j + 2, :])
        def xbf(t):
            return (xb_pre[t][:], xTt[t * P:(t + 1) * P, :]
                    .rearrange("p (kt c) -> p kt c", kt=KT))
        def x8f(t):
            return (x8_pre[t][:], x8t[t * P:(t + 1) * P, :]
                    .rearrange("p (kt c) -> p kt c", kt=KT))
        def sblk(t0, gt):
            st = spool.tile([P, P], BF16, tag="sblk")
            s_tiles[(t0, gt)] = st
            return (st[:], Sm[t0 * P:(t0 + 1) * P, gt * P:(gt + 1) * P])
        plan = {
            nc.sync:   [x8f(0), kwp(2), x8f(2), vwp(1), x8f(4), xbf(1)],
            nc.scalar: [kwp(0), kwp(3), x8f(3), vwp(2), vwp(3), x8f(5)],
            nc.gpsimd: [kwp(1), x8f(1), xbf(0), vwp(0), sblk(0, 0), sblk(1, 0)],
        }
        for eng, items in plan.items():
            for dst, srcap in items:
                eng.dma_start(dst, srcap)

        xb_tiles = dict(xb_pre)
        x8_tiles = dict(x8_pre)
        ksq_tiles = {}
        pending = None

        def emit_k(tk, x8):
            kps = ps_kv.tile([P, CLOC], F32, tag="kv", name="kps")
            for j in range(KT // 2):
                nc.tensor.matmul(kps[:], x8[:, 2 * j:2 * j + 2, :],
                                 kw_sb[:, 2 * j:2 * j + 2, :],
                                 perf_mode=mybir.MatmulPerfMode.DoubleRow,
                                 start=(j == 0),
                                 stop=(j == KT // 2 - 1 and not with_kb))
            if with_kb:
                nc.tensor.matmul(kps[:], ones_sb[:], kb_sb[:], start=False, stop=True)
            # square immediately: frees the PSUM slot early and decouples
            # the ||k|| chain from the k/v PE cadence
            ksq = smpool.tile([P, CLOC], F32, tag="ksq", bufs=LEAD + 2)
            nc.scalar.activation(ksq[:], kps[:], mybir.ActivationFunctionType.Square)
            ksq_tiles[tk] = ksq

        # k prologue: tiles 0..LEAD-1
        for tk in range(LEAD):
            emit_k(tk, x8_tiles.pop(tk))

        for t in range(TT):
            tk = t + LEAD
            # prefetches: x8 for tile t+LEAD+2, xb for tile t+2
            tn8 = t + LEAD + 2
            if tn8 < TT and tn8 not in x8_tiles:
                x8n = x8pool.tile([P, KT, P], FP8, tag="x8blk", bufs=LEAD + 3,
                                  name="x8")
                nc.sync.dma_start(x8n[:], x8t[tn8 * P:(tn8 + 1) * P, :]
                                  .rearrange("p (kt c) -> p kt c", kt=KT))
                x8_tiles[tn8] = x8n
            tn = t + 2
            if tn < TT and tn not in xb_tiles:
                xbn = xpool.tile([P, KT, P], BF16, tag="xblk", bufs=4, name="xb")
                nc.sync.dma_start(xbn[:], xTt[tn * P:(tn + 1) * P, :]
                                  .rearrange("p (kt c) -> p kt c", kt=KT))
                xb_tiles[tn] = xbn
            if t == 8:
                nc.scalar.dma_start(ct_sb[:], ctt.rearrange(
                    "p (h g) -> p h g", h=NHALF))
            if t == 12:
                nc.scalar.dma_start(ow_sb[:], owt.rearrange(
                    "p (gt e) -> p gt e", gt=GT))
            # prefetch S blocks two tiles ahead
            tp = t + 2
            if tp < TT:
                tsl = slice(tp * P, (tp + 1) * P)
                for gt in tile_gts[tp]:
                    if (tp, gt) not in s_tiles:
                        st = spool.tile([P, P], BF16, tag="sblk")
                        nc.gpsimd.dma_start(st[:], Sm[tsl, gt * P:(gt + 1) * P])
                        s_tiles[(tp, gt)] = st

            if tk < TT:
                emit_k(tk, x8_tiles.pop(tk))

            xb = xb_tiles.pop(t)
            vps = ps_kv.tile([P, CLOC], F32, tag="kv", name="vps")
            for kt in range(KT):
                nc.tensor.matmul(vps[:], xb[:, kt, :], vw_sb[:, kt, :],
                                 start=(kt == 0), stop=(kt == KT - 1 and not with_vb))
            if with_vb:
                nc.tensor.matmul(vps[:], ones_sb[:], vb_sb[:], start=False, stop=True)

            # scatter of the previous tile (its wv is ready by now)
            if pending is not None:
                emit_scatter(*pending)
                for kind, r, slot in jobs_at.get(pending[0], []):
                    if kind == "pre":
                        job_range_pre(r, slot)
                    else:
                        job_range(r)

            # ||k|| per head from the (already squared) k of tile t
            ksq = ksq_tiles.pop(t)
            km2 = smpool.tile([P, HLOC], F32, tag="km2")
            nc.vector.reduce_sum(km2[:], ksq[:].rearrange("p (h d) -> p h d", d=HD),
                                 axis=mybir.AxisListType.X)
            km = smpool.tile([P, HLOC], F32, tag="km")
            nc.scalar.sqrt(km[:], km2[:])

            # wv = v * ||k|| -> bf16
            wv = wvpool.tile([P, CLOC], BF16, tag="wv")
            nc.vector.tensor_tensor(
                wv[:].rearrange("p (h d) -> p h d", d=HD),
                vps[:].rearrange("p (h d) -> p h d", d=HD),
                km[:].unsqueeze(2).broadcast_to((P, HLOC, HD)),
                mybir.AluOpType.mult)
            pending = (t, wv)

        emit_scatter(*pending)

        def keep_warm(n):
            # dependency-free matmuls on long-resident operands into a
            # throwaway PSUM tile (kv ring: its tail slots are stale and
            # fully consumed): bridge tail-entry PE gaps so the PE stays
            # in its full-clock p-state for the fin matmuls
            wt = ps_kv.tile([P, CLOC], F32, tag="kv", name="warm")
            for i in range(n):
                nc.tensor.matmul(wt[:], field_sb[:, 0, 0:P],
                                 ct_sb[:, 0, 0:CLOC],
                                 start=(i == 0), stop=(i == n - 1))

        # interleave the two tail ranges: both ranges' conv matmuls issue
        # back-to-back so each range's DVE accumulate hides under the other
        # range's PE work; big range first so the drain-gating final DMA is
        # the small one.
        fins = sorted(tail_jobs, key=lambda rs: -(rs[0]["ghi"] - rs[0]["glo"]))
        keep_warm(4)   # bridges the h7 field-copy latency
        cvs = [(r, slot, conv_mms(r, [NHALF - 1])) for r, slot in fins]
        for r, slot, cv in cvs:
            fin_add(r, slot, cv)
        keep_warm(4)   # bridges the DVE accumulate latency
        for r, slot, cv in cvs:
            job_A(r)

    nc.compile()
    return nc


_PROGRAM_CACHE = {}
_PLANS_CACHE = {}


def _get_plans():
    if "p" not in _PLANS_CACHE:
        _PLANS_CACHE["p"] = _plans()
    return _PLANS_CACHE["p"]


def _get_program(with_kb, with_vb):
    key = (with_kb, with_vb)
    if key not in _PROGRAM_CACHE:
        _PROGRAM_CACHE[key] = _build_program(with_kb, with_vb, _get_plans())
    return _PROGRAM_CACHE[key]


def kernel(x, q_w, q_b, k_w, k_b, v_w, v_b, out_w, out_b):
    global LAST_RESULT
    x = np.asarray(x, dtype=np.float32)
    k_w = np.asarray(k_w, dtype=np.float32)
    k_b = np.asarray(k_b, dtype=np.float32)
    v_w = np.asarray(v_w, dtype=np.float32)
    v_b = np.asarray(v_b, dtype=np.float32)
    out_w = np.asarray(out_w, dtype=np.float32)
    out_b = np.asarray(out_b, dtype=np.float32)

    with_kb = bool(np.any(k_b))
    with_vb = bool(np.any(v_b))
    nc = _get_program(with_kb, with_vb)
    pl = _get_plans()
    Smat = pl["Smat"].astype(NP_BF16)
    # 1/KSCALE compensates the x KSCALE on the fp8 k-weights (exact: the
    # bf16 CT values just shift exponent by 5)
    CTm = pl["CTm"] * np.float32(1.0 / KSCALE)
    # ctt[p, h*G+g] = CTm[h*64+p, g]
    ctt = np.ascontiguousarray(
        CTm.reshape(NHALF, HB, G).transpose(1, 0, 2).reshape(HB, NHALF * G)
    ).astype(NP_BF16)

    in_maps = []
    for c in range(NCORES):
        b, hg = c // 2, c % 2
        chs = slice(hg * CLOC, (hg + 1) * CLOC)
        # xTt[t*128+p, kt*128+c] = x[b][t*128+c, kt*128+p]
        xb = x[b].reshape(TT, P, KT, P).transpose(0, 3, 2, 1) \
            .reshape(TT * P, KT * P)
        # kwt[p, kt*CLOC+ch] = KSCALE * k_w[chs][ch, kt*128+p]  (fp8)
        kwl = (k_w[chs, :].T * np.float32(KSCALE)) \
            .reshape(KT, P, CLOC).transpose(1, 0, 2).reshape(P, KT * CLOC)
        vwl = v_w[chs, :].T.reshape(KT, P, CLOC).transpose(1, 0, 2) \
            .reshape(P, KT * CLOC)
        # owt[p, ct*D+e] = out_w[e, ct*128+p(within chs)]
        owl = out_w[:, chs].T.reshape(GT, P, D).transpose(1, 0, 2) \
            .reshape(P, GT * D)
        m = {
            "xTt": np.ascontiguousarray(xb).astype(NP_BF16),
            "x8t": np.ascontiguousarray(xb).astype(NP_FP8),
            "kwt": np.ascontiguousarray(kwl).astype(NP_FP8),
            "vwt": np.ascontiguousarray(vwl).astype(NP_BF16),
            "owt": np.ascontiguousarray(owl).astype(NP_BF16),
            "ctt": ctt,
            "Smat": Smat,
        }
        if with_kb:
            # k is computed scaled by KSCALE on device; scale the bias too
            m["kb"] = np.ascontiguousarray(
                k_b[chs][None, :] * np.float32(KSCALE)).astype(NP_BF16)
        if with_vb:
            m["vb"] = np.ascontiguousarray(v_b[chs][None, :]).astype(NP_BF16)
        if with_kb or with_vb:
            m["ones"] = np.ones((1, P), dtype=NP_BF16)
        in_maps.append(m)

    res = run_bass_kernel_spmd(nc, in_maps, core_ids=list(range(NCORES)),
                               trace=TRACE)
    LAST_RESULT = res

    idx = pl["idx"]
    out = np.empty((B, N, D), dtype=np.float32)
    for b in range(B):
        # unshard: sum the two head-group partials of A, then replicate
        # bin rows out to tokens (pure gather) and add the output bias.
        A = np.zeros((G, D), dtype=np.float32)
        for part in (res.results[2 * b]["aout"], res.results[2 * b + 1]["aout"]):
            for r in pl["ranges"]:
                ri, glo, ghi = r["ri"], r["glo"], r["ghi"]
                A[glo:ghi] += part[ri * P:ri * P + (ghi - glo)]
        out[b] = A[idx]
        out[b] += out_b[None, :]
    return out
